# revision 10
# baseline (speedup 1.0000x reference)
"""Trainium2 Bass kernel for nn_BaselineOut (article/option additive-attention MRC head).

Contract: kernel(**inputs) takes FULL unsharded inputs (numpy), returns FULL
[32, 5] float32 logits.  Internally: data-parallel over batch across 8 cores
(4 batch items per core), all params replicated.

Math notes (vs reference):
  - oqc gather is done as a one-hot matmul on device (host only encodes the
    int indices as a one-hot fp32 matrix - a layout/encoding transform).
  - V-projection is pulled out of the attention sum by linearity:
        sum_l softmax_l * (V @ Vw^T + Vb) = (sum_l softmax_l * V) @ Vw^T + Vb
    so the [B*L,H]x[H,H] V matmul collapses to a weighted sum over L plus a
    tiny [B,H]x[H,H] matmul.
  - softmax logit bias (vb) is dropped: softmax is shift-invariant.
  - exp is computed without max-subtraction: |logit| <= ||vw||_1 ~ 36, and
    exp(36) is comfortably inside fp32 range.
  - Large matmuls run with float32r operands (full-rate fp32 on the PE).
"""

import functools
import sys

import numpy as np

sys.path.insert(0, "/opt/trn_rl_repo")

import concourse.bass as bass  # noqa: E402
from concourse import bacc  # noqa: E402
import concourse.tile as tile  # noqa: E402
from concourse import mybir  # noqa: E402
from concourse.bass import ds, ts  # noqa: E402

B, LA, LQ, LO, H, OUT = 32, 2048, 64, 32, 1024, 5
NCORES = 8
BL = B // NCORES  # 4 batch items per core
NOPT = 5
F32 = mybir.dt.float32
F32R = mybir.dt.float32r
LT = 512  # article l-tile (free dim of the big matmuls)
NLT = LA // LT  # 4
C = H // 128  # 8 h-chunks
BO = BL * NOPT  # 20 (b, option) pairs per core
AF = mybir.ActivationFunctionType
ALU = mybir.AluOpType
AX = mybir.AxisListType


def build_nc() -> bass.Bass:
    nc = bacc.Bacc("TRN2", target_bir_lowering=False, debug=False)

    # ---- DRAM I/O (per-core shard; names are the in_map keys) ----
    artT = nc.dram_tensor("artT", [BL, H, LA], F32, kind="ExternalInput").ap()
    optT = nc.dram_tensor("optT", [BL, H, NOPT, LO], F32, kind="ExternalInput").ap()
    qcd = nc.dram_tensor("qc", [BL, LQ, H], F32, kind="ExternalInput").ap()
    ohd = nc.dram_tensor("oh", [LQ, BL], F32, kind="ExternalInput").ap()
    wQa = nc.dram_tensor("aQwT", [H, H], F32, kind="ExternalInput").ap()
    wKa = nc.dram_tensor("aKwT", [H, H], F32, kind="ExternalInput").ap()
    wVa = nc.dram_tensor("aVwT", [H, H], F32, kind="ExternalInput").ap()
    wQd = nc.dram_tensor("dQwT", [H, H], F32, kind="ExternalInput").ap()
    wKd = nc.dram_tensor("dKwT", [H, H], F32, kind="ExternalInput").ap()
    wVd = nc.dram_tensor("dVwT", [H, H], F32, kind="ExternalInput").ap()
    vwad = nc.dram_tensor("vwaT", [128, C], F32, kind="ExternalInput").ap()
    vwdd = nc.dram_tensor("vwdT", [128, C], F32, kind="ExternalInput").ap()
    qkbd = nc.dram_tensor("qkbT", [128, C], F32, kind="ExternalInput").ap()
    dqkbd = nc.dram_tensor("dqkbT", [128, C], F32, kind="ExternalInput").ap()
    avbd = nc.dram_tensor("avbT", [128, C], F32, kind="ExternalInput").ap()
    dvbd = nc.dram_tensor("dvbT", [128, C], F32, kind="ExternalInput").ap()
    fwd = nc.dram_tensor("fwT", [128, NOPT, C, OUT], F32, kind="ExternalInput").ap()
    fbd = nc.dram_tensor("fb", [BL, OUT], F32, kind="ExternalInput").ap()
    onesd = nc.dram_tensor("ones1", [1, 128], F32, kind="ExternalInput").ap()
    outd = nc.dram_tensor("out", [BL, OUT], F32, kind="ExternalOutput").ap()

    with tile.TileContext(nc) as tc:
        with (
            tc.tile_pool(name="stream", bufs=3) as stream,
            tc.tile_pool(name="wbig", bufs=2) as wbig,
            tc.tile_pool(name="mpool", bufs=4) as mpool,
            tc.tile_pool(name="spool", bufs=4) as spool,
            tc.tile_pool(name="rpool", bufs=2) as rpool,
            tc.tile_pool(name="rdpool", bufs=1) as rdpool,
            tc.tile_pool(name="ubuf", bufs=2) as ubuf,
            tc.tile_pool(name="one", bufs=1) as one,
            tc.tile_pool(name="pacc", bufs=4, space="PSUM") as pacc,
            tc.tile_pool(name="prow", bufs=2, space="PSUM") as prow,
            tc.tile_pool(name="scratch", bufs=1) as scratch,
            tc.tile_pool(name="psml", bufs=2, space="PSUM") as psml,
        ):
            # ---------- small constant loads ----------
            vwa = one.tile([128, C], F32R, tag="vwa")
            nc.gpsimd.dma_start(out=vwa, in_=vwad)
            vwd = one.tile([128, C], F32R, tag="vwd")
            nc.gpsimd.dma_start(out=vwd, in_=vwdd)
            qkb = one.tile([128, C], F32, tag="qkb")
            nc.sync.dma_start(out=qkb, in_=qkbd)
            dqkb = one.tile([128, C], F32, tag="dqkb")
            nc.sync.dma_start(out=dqkb, in_=dqkbd)
            avb = one.tile([128, C], F32, tag="avb")
            nc.sync.dma_start(out=avb, in_=avbd)
            dvb = one.tile([128, C], F32, tag="dvb")
            nc.sync.dma_start(out=dvb, in_=dvbd)
            fw = one.tile([128, NOPT, C, OUT], F32, tag="fw")
            nc.sync.dma_start(out=fw, in_=fwd)
            fb = one.tile([BL, OUT], F32, tag="fb")
            nc.sync.dma_start(out=fb, in_=fbd)
            oht = one.tile([LQ, BL], F32, tag="oht")
            nc.sync.dma_start(out=oht, in_=ohd)
            ones = one.tile([1, 128], F32R, tag="ones")
            nc.gpsimd.dma_start(out=ones, in_=onesd)

            # ---------- weights (big, rotate through 3 slots) ----------
            wq = wbig.tile([128, C, H], F32, tag="w")
            nc.gpsimd.dma_start(out=wq, in_=wQa.rearrange("(c p) o -> p c o", p=128))
            wk = wbig.tile([128, C, H], F32R, tag="w")
            nc.gpsimd.dma_start(out=wk, in_=wKa.rearrange("(c p) o -> p c o", p=128))
            wv = wbig.tile([128, C, H], F32, tag="w")
            nc.gpsimd.dma_start(out=wv, in_=wVa.rearrange("(c p) o -> p c o", p=128))

            # ---------- gather oqc via one-hot matmul ----------
            qct = stream.tile([LQ, BL, H], F32, tag="stream")
            for b in range(BL):
                nc.sync.dma_start(out=qct[:, b, :], in_=qcd[b])
            oqcT = one.tile([128, C, BL], F32, tag="oqcT")
            for c in range(C):
                po = psml.tile([128, BL], F32, tag="sml")
                for b in range(BL):
                    nc.tensor.matmul(
                        po[:, b : b + 1],
                        lhsT=qct[:, b, ts(c, 128)],
                        rhs=oht[:, b : b + 1],
                        start=True,
                        stop=True,
                    )
                nc.vector.tensor_copy(oqcT[:, c, :], po)

            # ---------- Qp^T = aQw @ oqc^T ; article tanh bias ----------
            biasA = one.tile([128, C, BL], F32, tag="biasA")
            for co in range(C):
                pq = psml.tile([128, BL], F32, tag="sml")
                for ci in range(C):
                    nc.tensor.matmul(
                        pq,
                        lhsT=wq[:, ci, ts(co, 128)],
                        rhs=oqcT[:, ci, :],
                        start=(ci == 0),
                        stop=(ci == C - 1),
                    )
                nc.vector.tensor_scalar_add(biasA[:, co, :], pq, qkb[:, co : co + 1])

            # ---------- article branch ----------
            s_sums = one.tile([1, BL, NLT], F32, tag="s_sums")
            uTun = one.tile([128, C, BL], F32, tag="uTun")
            for b in range(BL):
                upart = ubuf.tile([128, C, NLT], F32, tag="upart")
                for lt in range(NLT):
                    T = stream.tile([128, C, LT], F32R, tag="stream")
                    nc.gpsimd.dma_start(
                        out=T,
                        in_=artT[b, :, ds(lt * LT, LT)].rearrange(
                            "(c p) l -> p c l", p=128
                        ),
                    )
                    lg = prow.tile([1, LT], F32, tag="lg")
                    for co in range(C):
                        kp = pacc.tile([128, LT], F32, tag="acc")
                        for ci in range(C):
                            nc.tensor.matmul(
                                kp,
                                lhsT=wk[:, ci, ts(co, 128)],
                                rhs=T[:, ci, :],
                                start=(ci == 0),
                                stop=(ci == C - 1),
                            )
                        mt = mpool.tile([128, LT], F32R, tag="mt")
                        nc.scalar.activation(
                            mt, kp, AF.Tanh, bias=biasA[:, co, b : b + 1]
                        )
                        nc.tensor.matmul(
                            lg,
                            lhsT=vwa[:, co : co + 1],
                            rhs=mt,
                            start=(co == 0),
                            stop=(co == C - 1),
                        )
                    st = spool.tile([1, LT], F32R, tag="st")
                    nc.scalar.activation(
                        st, lg, AF.Exp, accum_out=s_sums[:, b, lt : lt + 1]
                    )
                    # replicate s~ across partitions: ones^T (x) st via PE
                    prep = pacc.tile([128, LT], F32, tag="acc")
                    nc.tensor.matmul(
                        prep,
                        lhsT=ones,
                        rhs=st,
                        start=True,
                        stop=True,
                    )
                    srep = rpool.tile([128, LT], F32, tag="srep")
                    nc.scalar.copy(srep, prep)
                    scr = scratch.tile([128, C, LT], F32, tag="scr")
                    nc.vector.tensor_mul(
                        scr,
                        T.bitcast(F32),
                        srep.unsqueeze(1).broadcast_to((128, C, LT)),
                    )
                    nc.vector.tensor_reduce(
                        upart[:, :, lt : lt + 1], scr, axis=AX.X, op=ALU.add
                    )
                # sum the NLT partial weighted sums -> unnormalized u^T
                nc.vector.tensor_reduce(
                    uTun[:, :, b : b + 1], upart, axis=AX.X, op=ALU.add
                )

            # normalization factors: 1/sum(exp) per b, replicated to 128 parts
            ssb = one.tile([1, BL], F32, tag="ssb")
            nc.vector.tensor_reduce(ssb, s_sums, axis=AX.X, op=ALU.add)
            psb = psml.tile([128, BL], F32, tag="sml")
            nc.tensor.matmul(psb, lhsT=ones.bitcast(F32), rhs=ssb, start=True, stop=True)
            rs_rep = one.tile([128, BL], F32, tag="rs_rep")
            nc.vector.reciprocal(rs_rep, psb)

            uT = one.tile([128, C, BL], F32, tag="uT")
            for b in range(BL):
                nc.vector.tensor_scalar_mul(
                    uT[:, :, b], uTun[:, :, b], rs_rep[:, b : b + 1]
                )

            # ---------- aq^T = aVw @ u^T + aVb ; option tanh bias ----------
            wdq = wbig.tile([128, C, H], F32, tag="w")
            nc.gpsimd.dma_start(out=wdq, in_=wQd.rearrange("(c p) o -> p c o", p=128))
            aqT = one.tile([128, C, BL], F32, tag="aqT")
            for co in range(C):
                pa = psml.tile([128, BL], F32, tag="sml")
                for ci in range(C):
                    nc.tensor.matmul(
                        pa,
                        lhsT=wv[:, ci, ts(co, 128)],
                        rhs=uT[:, ci, :],
                        start=(ci == 0),
                        stop=(ci == C - 1),
                    )
                nc.vector.tensor_scalar_add(aqT[:, co, :], pa, avb[:, co : co + 1])

            biasO = one.tile([128, C, BL], F32, tag="biasO")
            for co in range(C):
                pq2 = psml.tile([128, BL], F32, tag="sml")
                for ci in range(C):
                    nc.tensor.matmul(
                        pq2,
                        lhsT=wdq[:, ci, ts(co, 128)],
                        rhs=aqT[:, ci, :],
                        start=(ci == 0),
                        stop=(ci == C - 1),
                    )
                nc.vector.tensor_scalar_add(biasO[:, co, :], pq2, dqkb[:, co : co + 1])

            # ---------- options branch ----------
            wdk = wbig.tile([128, C, H], F32R, tag="w")
            nc.gpsimd.dma_start(out=wdk, in_=wKd.rearrange("(c p) o -> p c o", p=128))
            OT = stream.tile([128, C, BL, NOPT, LO], F32R, tag="stream")
            for b in range(BL):
                nc.gpsimd.dma_start(
                    out=OT[:, :, b],
                    in_=optT[b].rearrange("(c p) o l -> p c o l", p=128),
                )
            mdt = stream.tile([128, C, BL, NOPT, LO], F32R, tag="stream")
            HALF = 2 * NOPT * LO  # 320 columns (2 batch items)
            for co in range(C):
                for h in range(2):
                    kpd = pacc.tile([128, HALF], F32, tag="acc")
                    for ci in range(C):
                        nc.tensor.matmul(
                            kpd,
                            lhsT=wdk[:, ci, ts(co, 128)],
                            rhs=OT[:, ci, ds(2 * h, 2)],
                            start=(ci == 0),
                            stop=(ci == C - 1),
                        )
                    for bq in range(2):
                        b = 2 * h + bq
                        nc.scalar.activation(
                            mdt[:, co, b],
                            kpd[:, ds(bq * NOPT * LO, NOPT * LO)],
                            AF.Tanh,
                            bias=biasO[:, co, b : b + 1],
                        )

            s_d = one.tile([1, BO * LO], F32, tag="s_d")
            for h in range(2):
                lgd = prow.tile([1, HALF], F32, tag="lg")
                for co in range(C):
                    nc.tensor.matmul(
                        lgd,
                        lhsT=vwd[:, co : co + 1],
                        rhs=mdt[:, co, ds(2 * h, 2)],
                        start=(co == 0),
                        stop=(co == C - 1),
                    )
                nc.scalar.activation(s_d[:, ds(h * HALF, HALF)], lgd, AF.Exp)

            sums_d = one.tile([1, BO], F32, tag="sums_d")
            nc.vector.tensor_reduce(
                sums_d,
                s_d.rearrange("p (bo l) -> p bo l", l=LO),
                axis=AX.X,
                op=ALU.add,
            )
            rec_d = one.tile([1, BO], F32, tag="rec_d")
            nc.vector.reciprocal(rec_d, sums_d)
            snd = one.tile([1, BO * LO], F32R, tag="snd")
            for bo in range(BO):
                nc.scalar.activation(
                    snd[:, ds(bo * LO, LO)],
                    s_d[:, ds(bo * LO, LO)],
                    AF.Copy,
                    scale=rec_d[:, bo : bo + 1],
                )
            sdrep = rdpool.tile([128, BO * LO], F32, tag="sdrep")
            for h in range(2):
                prepd = pacc.tile([128, HALF], F32, tag="acc")
                nc.tensor.matmul(
                    prepd,
                    lhsT=ones,
                    rhs=snd[:, ds(h * HALF, HALF)],
                    start=True,
                    stop=True,
                )
                nc.scalar.copy(sdrep[:, ds(h * HALF, HALF)], prepd)

            wdv = wbig.tile([128, C, H], F32, tag="w")
            nc.gpsimd.dma_start(out=wdv, in_=wVd.rearrange("(c p) o -> p c o", p=128))

            u_dT = one.tile([128, C, BO], F32, tag="u_dT")
            OTf = OT.bitcast(F32).rearrange("p c b o l -> p c (b o) l")
            sdv = sdrep.rearrange("p (bo l) -> p bo l", l=LO)
            for c in range(C):
                scrd = scratch.tile([128, BO, LO], F32, tag="scr")
                nc.vector.tensor_mul(scrd, OTf[:, c], sdv)
                nc.vector.tensor_reduce(
                    u_dT[:, c : c + 1, :].rearrange("p one bo -> p bo one"),
                    scrd,
                    axis=AX.X,
                    op=ALU.add,
                )

            featsT = one.tile([128, C, BO], F32, tag="featsT")
            for co in range(C):
                pf = psml.tile([128, BO], F32, tag="sml")
                for ci in range(C):
                    nc.tensor.matmul(
                        pf,
                        lhsT=wdv[:, ci, ts(co, 128)],
                        rhs=u_dT[:, ci, :],
                        start=(ci == 0),
                        stop=(ci == C - 1),
                    )
                nc.vector.tensor_scalar_add(featsT[:, co, :], pf, dvb[:, co : co + 1])

            # ---------- final linear: out[b, :] = feats[b] @ f_w^T + f_b ----------
            pout = psml.tile([BL, OUT], F32, tag="sml")
            fT = featsT.rearrange("p c (b o) -> p c b o", o=NOPT)
            n = 0
            for c in range(C):
                for o in range(NOPT):
                    nc.tensor.matmul(
                        pout,
                        lhsT=fT[:, c, :, o],
                        rhs=fw[:, o, c, :],
                        start=(n == 0),
                        stop=(n == C * NOPT - 1),
                    )
                    n += 1
            out_s = one.tile([BL, OUT], F32, tag="out_s")
            nc.vector.tensor_add(out_s, pout, fb)
            nc.sync.dma_start(out=outd, in_=out_s)

    nc.compile()
    return nc


@functools.lru_cache(maxsize=1)
def get_nc() -> bass.Bass:
    return build_nc()


def make_in_maps(inputs: dict) -> list[dict]:
    art = np.ascontiguousarray(np.asarray(inputs["article_contexts"], np.float32))
    qc = np.ascontiguousarray(np.asarray(inputs["question_contexts"], np.float32))
    opt = np.ascontiguousarray(np.asarray(inputs["options_embeds"], np.float32))
    idx = np.asarray(inputs["answer_indices"]).astype(np.int64)

    def wT(name):
        return np.ascontiguousarray(np.asarray(inputs[name], np.float32).T)

    aQwT, aKwT, aVwT = wT("a_Qw"), wT("a_Kw"), wT("a_Vw")
    dQwT, dKwT, dVwT = wT("d_Qw"), wT("d_Kw"), wT("d_Vw")

    def colvec(v):  # [H] -> [128, C] with chunk-major free dim
        return np.ascontiguousarray(
            np.asarray(v, np.float32).reshape(C, 128).T
        )

    vwaT = colvec(np.asarray(inputs["a_vw"], np.float32).reshape(H))
    vwdT = colvec(np.asarray(inputs["d_vw"], np.float32).reshape(H))
    qkbT = colvec(
        np.asarray(inputs["a_Qb"], np.float32) + np.asarray(inputs["a_Kb"], np.float32)
    )
    dqkbT = colvec(
        np.asarray(inputs["d_Qb"], np.float32) + np.asarray(inputs["d_Kb"], np.float32)
    )
    avbT = colvec(np.asarray(inputs["a_Vb"], np.float32))
    dvbT = colvec(np.asarray(inputs["d_Vb"], np.float32))
    f_w = np.asarray(inputs["f_w"], np.float32)  # [OUT, 5H]
    fwT = np.ascontiguousarray(
        f_w.reshape(OUT, NOPT, C, 128).transpose(3, 1, 2, 0)
    )  # [128, o, c, OUT]
    f_b = np.asarray(inputs["f_b"], np.float32).reshape(1, OUT)

    artT = np.ascontiguousarray(art.transpose(0, 2, 1))  # [B, H, LA]
    optT = np.ascontiguousarray(opt.transpose(0, 3, 1, 2))  # [B, H, 5, LO]
    onehot = np.zeros((B, LQ), np.float32)
    onehot[np.arange(B), idx] = 1.0

    shared = dict(
        aQwT=aQwT, aKwT=aKwT, aVwT=aVwT, dQwT=dQwT, dKwT=dKwT, dVwT=dVwT,
        vwaT=vwaT, vwdT=vwdT, qkbT=qkbT, dqkbT=dqkbT, avbT=avbT, dvbT=dvbT,
        fwT=fwT, fb=np.ascontiguousarray(np.tile(f_b, (BL, 1))),
        ones1=np.ones((1, 128), np.float32),
    )
    in_maps = []
    for r in range(NCORES):
        s = slice(r * BL, (r + 1) * BL)
        m = dict(shared)
        m["artT"] = artT[s]
        m["optT"] = optT[s]
        m["qc"] = qc[s]
        m["oh"] = np.ascontiguousarray(onehot[s].T)
        in_maps.append(m)
    return in_maps


def run(inputs: dict, trace: bool = False, tmpdir=None):
    from concourse.bass_utils import run_bass_kernel_spmd

    nc = get_nc()
    in_maps = make_in_maps(inputs)
    res = run_bass_kernel_spmd(
        nc, in_maps, core_ids=list(range(NCORES)), trace=trace, tmpdir=tmpdir
    )
    out = np.concatenate([res.results[r]["out"] for r in range(NCORES)], axis=0)
    return out, res


def kernel(**inputs) -> np.ndarray:
    out, _ = run(inputs, trace=False)
    return out
